# revision 1
# baseline (speedup 1.0000x reference)
"""MoE block (B=2,S=2048,D=2048,FF=8192,E=16,K=2,C=640) on 8 trn2 cores.

v2: fully core-local routing (no collectives). Each core computes the exact
fp32 gate via a 3-term bf16 residual decomposition of x@Wg, streamed in
512-token chunks so top-2/positions/slot-scatter pipeline behind the DMA.
Slot-0/slot-1 tables are scattered incrementally (running block-prefix
counts) and merged at gather time with a count-shifted indirect gather.
Expert FFN in bf16 (fp32 accum); raw expert outputs Ye + (token,gate) slot
maps are written out and the gate-weighted combine + b2 bias happens on
host (expert-parallel unshard).
"""
import sys
sys.path.insert(0, "/opt/trn_rl_repo")
import numpy as np
import ml_dtypes

import concourse.bass as bass
import concourse.mybir as mybir
import concourse.tile as tile
from concourse import bacc
from concourse.bass_utils import run_bass_kernel_spmd

F32 = mybir.dt.float32
BF16 = mybir.dt.bfloat16
I32 = mybir.dt.int32
U32 = mybir.dt.uint32
AL = mybir.AluOpType
ACTF = mybir.ActivationFunctionType

B, S, D, FF, E, K = 2, 2048, 2048, 8192, 16, 2
T = B * S                 # 4096 tokens
C = 640                   # per-expert capacity
NB = T // 128             # 32 token blocks
EL = 2                    # local experts per core
NF = FF // 128            # 64 f-tiles
ND = D // 512             # 4 dd chunks
NCT = C // 128            # 5 capacity tiles per expert
NK = D // 128             # 16 contraction tiles of D
NCH = 8                   # gate token chunks (512 tokens each)
BPC = 4                   # blocks per chunk
SENT = float(T)           # sentinel token id -> zero row of xb

_CACHE = {}


def _build_nc():
    nc = bacc.Bacc(None, target_bir_lowering=False, debug=True)

    x8t = nc.dram_tensor("x8t", [NCH, 128, NK, 512], BF16, kind="ExternalInput")
    xrt = nc.dram_tensor("xrt", [NCH, 128, NK, 512], BF16, kind="ExternalInput")
    xb = nc.dram_tensor("xb", [T + 1, D], BF16, kind="ExternalInput")
    wg8h = nc.dram_tensor("wg8h", [128, NK * E], BF16, kind="ExternalInput")
    wgrh = nc.dram_tensor("wgrh", [128, NK * E], BF16, kind="ExternalInput")
    w1h = nc.dram_tensor("w1h", [EL, NF, 128, NK * 128], BF16, kind="ExternalInput")
    w2h = nc.dram_tensor("w2h", [EL, ND, NF, 128, 512], BF16, kind="ExternalInput")
    b1h = nc.dram_tensor("b1h", [EL, 128, NF], F32, kind="ExternalInput")
    tokids = nc.dram_tensor("tokids", [128, NB], F32, kind="ExternalInput")
    iota16 = nc.dram_tensor("iota16", [128, E], F32, kind="ExternalInput")
    iotac = nc.dram_tensor("iotac", [128, NCT], F32, kind="ExternalInput")
    tri128 = nc.dram_tensor("tri128", [128, 128], BF16, kind="ExternalInput")
    ident16 = nc.dram_tensor("ident16", [16, 16], F32, kind="ExternalInput")
    onescol = nc.dram_tensor("onescol", [128, 1], BF16, kind="ExternalInput")
    onesrow = nc.dram_tensor("onesrow", [1, 128], F32, kind="ExternalInput")
    identb = nc.dram_tensor("identb", [128, 128], BF16, kind="ExternalInput")
    basev = nc.dram_tensor("basev", [128, 1], F32, kind="ExternalInput")
    esel2 = nc.dram_tensor("esel2", [128, EL * E], F32, kind="ExternalInput")

    s0t = nc.dram_tensor("s0t", [EL * C, 2], F32)
    s1t = nc.dram_tensor("s1t", [EL * C, 2], F32)

    yeh = nc.dram_tensor("yeh", [EL, NCT, 128, ND * 512], F32, kind="ExternalOutput")
    gsl = nc.dram_tensor("gsl", [EL, 128, NCT, 2], F32, kind="ExternalOutput")

    with tile.TileContext(nc) as tc:
        with tc.tile_pool(name="consts", bufs=1) as cp:
            wg8_sb = cp.tile([128, NK * E], BF16)
            nc.sync.dma_start(wg8_sb[:], wg8h[:])
            wgr_sb = cp.tile([128, NK * E], BF16)
            nc.sync.dma_start(wgr_sb[:], wgrh[:])
            iota_sb = cp.tile([128, E], F32)
            iotac_sb = cp.tile([128, NCT], F32)
            tri_sb = cp.tile([128, 128], BF16)
            i16_sb = cp.tile([16, 16], F32)
            ones_sb = cp.tile([128, 1], BF16)
            onesr_sb = cp.tile([1, 128], F32)
            idb_sb = cp.tile([128, 128], BF16)
            base_sb = cp.tile([128, 1], F32)
            tok_sb = cp.tile([128, NB], F32)
            esel_sb = cp.tile([128, EL * E], F32)
            b1_sb = [cp.tile([128, NF], F32, tag=f"b1_{e}", name=f"b1_{e}")
                     for e in range(EL)]
            cnt0e = [cp.tile([128, 1], F32, tag=f"cnt0e{e}", name=f"cnt0e{e}")
                     for e in range(EL)]
            sent = cp.tile([128, EL * NCT, 2], F32)

            # ---------------- routing (fully local, chunk-pipelined) -------
            with tc.tile_pool(name="rout", bufs=1) as rp, \
                 tc.tile_pool(name="psr", bufs=1, space="PSUM") as pr:
                # prefetch the first two gate chunks ahead of the small
                # const DMAs so chunk-0 compute starts as early as possible
                pre = []
                for c in range(2):
                    p8 = rp.tile([128, NK, 512], BF16, tag="x8c", bufs=2)
                    nc.sync.dma_start(p8[:], x8t[c])
                    prr = rp.tile([128, NK, 512], BF16, tag="xrc", bufs=2)
                    nc.scalar.dma_start(prr[:], xrt[c])
                    pre.append((p8, prr))
                # consts (emitted after the prefetch so the big chunk DMAs
                # lead both hardware DGE queues)
                nc.scalar.dma_start(iota_sb[:], iota16[:])
                nc.scalar.dma_start(iotac_sb[:], iotac[:])
                nc.scalar.dma_start(tri_sb[:], tri128[:])
                nc.scalar.dma_start(i16_sb[:], ident16[:])
                nc.scalar.dma_start(ones_sb[:], onescol[:])
                nc.scalar.dma_start(onesr_sb[:], onesrow[:])
                nc.scalar.dma_start(idb_sb[:], identb[:])
                nc.scalar.dma_start(base_sb[:], basev[:])
                nc.scalar.dma_start(tok_sb[:], tokids[:])
                nc.scalar.dma_start(esel_sb[:], esel2[:])
                for e in range(EL):
                    nc.scalar.dma_start(b1_sb[e][:], b1h[e])
                # sentinel-init both slot tables: tok=T (zero row), gate=0
                nc.vector.memset(sent[:, :, 0:1], SENT)
                nc.vector.memset(sent[:, :, 1:2], 0.0)
                nc.scalar.dma_start(s0t[:].rearrange("(n p) c -> p n c", p=128),
                                    sent[:])
                nc.scalar.dma_start(s1t[:].rearrange("(n p) c -> p n c", p=128),
                                    sent[:])
                logits = rp.tile([128, NB, E], F32)
                mx = rp.tile([128, NB, 8], F32)
                mi = rp.tile([128, NB, 8], U32)
                oh0 = rp.tile([128, NB, E], BF16)
                oh1 = rp.tile([128, NB, E], BF16)
                i0f = rp.tile([128, NB], F32)
                i1f = rp.tile([128, NB], F32)
                g0 = rp.tile([128, NB], F32)
                g1 = rp.tile([128, NB], F32)
                dte = rp.tile([128, NB], F32)
                exd = rp.tile([128, NB], F32)
                den = rp.tile([128, NB], F32)
                pos = [rp.tile([128, NB], F32, tag=f"pos{s}", name=f"pos{s}")
                       for s in range(2)]
                pay = [rp.tile([128, NB, 2], F32, tag=f"pay{s}", name=f"pay{s}")
                       for s in range(2)]
                offi = [rp.tile([128, NB], I32, tag=f"offi{s}", name=f"offi{s}")
                        for s in range(2)]
                run = [rp.tile([1, E], F32, tag=f"run{s}", name=f"run{s}")
                       for s in range(2)]
                for s in range(2):
                    nc.vector.memset(run[s][:], 0.0)
                    nc.vector.tensor_copy(pay[s][:, :, 0], tok_sb[:])

                for c in range(NCH):
                    if c < 2:
                        x8c, xrc = pre[c]
                    else:
                        x8c = rp.tile([128, NK, 512], BF16, tag="x8c", bufs=2)
                        nc.sync.dma_start(x8c[:], x8t[c])
                        xrc = rp.tile([128, NK, 512], BF16, tag="xrc", bufs=2)
                        nc.scalar.dma_start(xrc[:], xrt[c])
                    glog = pr.tile([16, 512], F32, tag="glog", bufs=2)
                    nmm = 3 * NK
                    im = 0
                    for wsb, xc in ((wg8_sb, x8c), (wg8_sb, xrc), (wgr_sb, x8c)):
                        for k in range(NK):
                            nc.tensor.matmul(glog[:], lhsT=wsb[:, k * E:(k + 1) * E],
                                             rhs=xc[:, k, :], start=(im == 0),
                                             stop=(im == nmm - 1))
                            im += 1
                    lgs = rp.tile([16, 512], F32, tag="lgs", bufs=2)
                    nc.vector.tensor_copy(lgs[:], glog[:])
                    for j in range(BPC):
                        b = BPC * c + j
                        tp16 = pr.tile([128, 16], F32, tag="tp16", bufs=1)
                        nc.tensor.transpose(out=tp16[:],
                                            in_=lgs[:, j * 128:(j + 1) * 128],
                                            identity=i16_sb[:])
                        nc.vector.tensor_copy(logits[:, b, :], tp16[:])
                        nc.vector.max(out=mx[:, b, :], in_=logits[:, b, :])
                        nc.vector.max_index(out=mi[:, b, :], in_max=mx[:, b, :],
                                            in_values=logits[:, b, :])
                    bs = slice(BPC * c, BPC * c + BPC)
                    nc.vector.tensor_copy(i0f[:, bs], mi[:, bs, 0])
                    nc.vector.tensor_copy(i1f[:, bs], mi[:, bs, 1])
                    for j in range(BPC):
                        b = BPC * c + j
                        nc.vector.tensor_tensor(
                            out=oh0[:, b, :], in0=iota_sb[:],
                            in1=i0f[:, b:b + 1].to_broadcast([128, E]),
                            op=AL.is_equal)
                        nc.vector.tensor_tensor(
                            out=oh1[:, b, :], in0=iota_sb[:],
                            in1=i1f[:, b:b + 1].to_broadcast([128, E]),
                            op=AL.is_equal)
                    # gates from top-2 logits: g0 = 1/(1+e^(l1-l0)), g1 = 1-g0
                    nc.vector.tensor_tensor(out=dte[:, bs], in0=mx[:, bs, 1],
                                            in1=mx[:, bs, 0], op=AL.subtract)
                    nc.scalar.activation(exd[:, bs], dte[:, bs], ACTF.Exp)
                    nc.vector.tensor_scalar_add(den[:, bs], exd[:, bs], 1.0)
                    nc.vector.reciprocal(g0[:, bs], den[:, bs])
                    nc.vector.tensor_tensor(out=g1[:, bs], in0=exd[:, bs],
                                            in1=g0[:, bs], op=AL.mult)
                    nc.vector.tensor_copy(pay[0][:, bs, 1], g0[:, bs])
                    nc.vector.tensor_copy(pay[1][:, bs, 1], g1[:, bs])

                    for s, (oh, idxf) in enumerate(((oh0, i0f), (oh1, i1f))):
                        # in-block inclusive cumsum via triangular matmul
                        cu = pr.tile([128, BPC, E], F32, tag="cu", bufs=2)
                        for j in range(BPC):
                            b = BPC * c + j
                            nc.tensor.matmul(cu[:, j, :], lhsT=tri_sb[:],
                                             rhs=oh[:, b, :], start=True, stop=True)
                        # per-(expert, block) counts of this chunk
                        cnp = pr.tile([1, BPC, E], F32, tag="cnp", bufs=1)
                        nc.tensor.matmul(
                            cnp[:].rearrange("o j e -> o (j e)"), lhsT=ones_sb[:],
                            rhs=oh[:, bs, :].rearrange("p j e -> p (j e)"),
                            start=True, stop=True)
                        cnr = rp.tile([1, BPC, E], F32, tag=f"cnr{s}", bufs=2)
                        nc.vector.tensor_copy(cnr[:], cnp[:])
                        # exclusive block offsets = running + in-chunk prefix
                        exc = rp.tile([1, BPC, E], F32, tag=f"exc{s}", bufs=2)
                        nc.vector.tensor_copy(exc[:, 0, :], run[s][:])
                        for j in range(1, BPC):
                            nc.vector.tensor_tensor(out=exc[:, j, :],
                                                    in0=exc[:, j - 1, :],
                                                    in1=cnr[:, j - 1, :], op=AL.add)
                        nc.vector.tensor_tensor(out=run[s][:], in0=exc[:, BPC - 1, :],
                                                in1=cnr[:, BPC - 1, :], op=AL.add)
                        # partition-broadcast via 1-partition ones matmul on PE
                        bcps = pr.tile([128, BPC, E], F32, tag="bcps", bufs=2)
                        nc.tensor.matmul(
                            bcps[:].rearrange("p j e -> p (j e)"),
                            lhsT=onesr_sb[:],
                            rhs=exc[:].rearrange("o j e -> o (j e)"),
                            start=True, stop=True)
                        bcs = rp.tile([128, BPC, E], F32, tag="bcs", bufs=2)
                        nc.vector.tensor_copy(bcs[:], bcps[:])
                        # position = (cu + bc) * oh summed over e, minus 1
                        t1 = rp.tile([128, BPC, E], F32, tag=f"t1{s}", bufs=2)
                        nc.vector.tensor_tensor(out=t1[:], in0=cu[:], in1=bcs[:],
                                                op=AL.add)
                        nc.vector.tensor_tensor(out=t1[:], in0=t1[:],
                                                in1=oh[:, bs, :], op=AL.mult)
                        for j in range(BPC):
                            b = BPC * c + j
                            nc.vector.tensor_reduce(out=pos[s][:, b:b + 1],
                                                    in_=t1[:, j, :],
                                                    axis=mybir.AxisListType.X,
                                                    op=AL.add)
                        nc.vector.tensor_scalar_add(pos[s][:, bs], pos[s][:, bs], -1.0)
                        # slot-table offset with local-range + capacity masks
                        offc = rp.tile([128, BPC], F32, tag=f"offc{s}", bufs=2)
                        m1 = rp.tile([128, BPC], F32, tag=f"m1s{s}", bufs=2)
                        m2 = rp.tile([128, BPC], F32, tag=f"m2s{s}", bufs=2)
                        nc.vector.tensor_scalar_mul(offc[:], idxf[:, bs], float(C))
                        nc.vector.tensor_tensor(out=offc[:], in0=offc[:],
                                                in1=pos[s][:, bs], op=AL.add)
                        nc.vector.tensor_scalar_sub(offc[:], offc[:], base_sb[:, 0:1])
                        nc.vector.tensor_scalar(m1[:], offc[:], 0.0, None, op0=AL.is_ge)
                        nc.vector.tensor_scalar(m2[:], offc[:], float(EL * C), None,
                                                op0=AL.is_lt)
                        nc.vector.tensor_tensor(out=m1[:], in0=m1[:], in1=m2[:],
                                                op=AL.mult)
                        nc.vector.tensor_scalar(m2[:], pos[s][:, bs], float(C), None,
                                                op0=AL.is_lt)
                        nc.vector.tensor_tensor(out=m1[:], in0=m1[:], in1=m2[:],
                                                op=AL.mult)
                        nc.vector.tensor_tensor(out=offc[:], in0=offc[:], in1=m1[:],
                                                op=AL.mult)
                        nc.vector.tensor_scalar(m2[:], m1[:], -2.0e9, 2.0e9,
                                                op0=AL.mult, op1=AL.add)
                        nc.vector.tensor_tensor(out=offc[:], in0=offc[:], in1=m2[:],
                                                op=AL.add)
                        nc.vector.tensor_copy(offi[s][:, bs], offc[:])
                        st = s0t if s == 0 else s1t
                        for j in range(BPC):
                            b = BPC * c + j
                            nc.gpsimd.indirect_dma_start(
                                out=st[:, :],
                                out_offset=bass.IndirectOffsetOnAxis(
                                    ap=offi[s][:, b:b + 1], axis=0),
                                in_=pay[s][:, b, :], in_offset=None,
                                bounds_check=EL * C - 1, oob_is_err=False)

                # total slot-0 counts per expert, selected for local experts
                c0ps = pr.tile([128, E], F32, tag="bcps", bufs=2)
                nc.tensor.matmul(c0ps[:], lhsT=onesr_sb[:], rhs=run[0][:],
                                 start=True, stop=True)
                c0b = rp.tile([128, E], F32)
                nc.vector.tensor_copy(c0b[:], c0ps[:])
                ct0t = rp.tile([128, E], F32, tag="ct0t", bufs=2)
                for e in range(EL):
                    nc.vector.tensor_tensor(out=ct0t[:], in0=c0b[:],
                                            in1=esel_sb[:, e * E:(e + 1) * E],
                                            op=AL.mult)
                    nc.vector.tensor_reduce(out=cnt0e[e][:], in_=ct0t[:],
                                            axis=mybir.AxisListType.X, op=AL.add)

            # ---------------- expert FFN + store ----------------
            with tc.tile_pool(name="ffn", bufs=1) as fp, \
                 tc.tile_pool(name="psf", bufs=1, space="PSUM") as pf:
                for e in range(EL):
                    # merge slot-1 into slot-0 table with count-shifted gather
                    mrg = fp.tile([128, NCT, 2], F32, tag=f"mrg{e}", name=f"mrg{e}")
                    nc.scalar.dma_start(
                        mrg[:], s0t[e * C:(e + 1) * C, :]
                        .rearrange("(ct p) c -> p ct c", p=128))
                    tm = fp.tile([128, NCT], F32, tag="tm", bufs=2)
                    mm1 = fp.tile([128, NCT], F32, tag="mmk", bufs=2)
                    nc.vector.tensor_scalar_sub(tm[:], iotac_sb[:], cnt0e[e][:, 0:1])
                    nc.vector.tensor_scalar(mm1[:], tm[:], 0.0, None, op0=AL.is_ge)
                    nc.vector.tensor_scalar_add(tm[:], tm[:], float(e * C))
                    nc.vector.tensor_tensor(out=tm[:], in0=tm[:], in1=mm1[:],
                                            op=AL.mult)
                    nc.vector.tensor_scalar(mm1[:], mm1[:], -2.0e9, 2.0e9,
                                            op0=AL.mult, op1=AL.add)
                    nc.vector.tensor_tensor(out=tm[:], in0=tm[:], in1=mm1[:],
                                            op=AL.add)
                    offm = fp.tile([128, NCT], I32, tag=f"offm{e}", name=f"offm{e}")
                    nc.vector.tensor_copy(offm[:], tm[:])
                    for ct in range(NCT):
                        nc.gpsimd.indirect_dma_start(
                            out=mrg[:, ct, :], out_offset=None, in_=s1t[:, :],
                            in_offset=bass.IndirectOffsetOnAxis(
                                ap=offm[:, ct:ct + 1], axis=0),
                            bounds_check=EL * C - 1, oob_is_err=False)
                    nc.scalar.dma_start(gsl[e], mrg[:])
                    tok_i = fp.tile([128, NCT], I32, tag=f"toki{e}", name=f"toki{e}")
                    nc.vector.tensor_copy(tok_i[:], mrg[:, :, 0])

                    # dispatch: gather token rows; transpose to [d, c] split
                    # across PE (ct 0-1) and XBAR DMA (ct 2-4) in parallel
                    xte = fp.tile([128, NK, C], BF16, tag=f"xte{e}", name=f"xte{e}")
                    for ct in (2, 0, 3, 1, 4):
                        xg = fp.tile([128, D], BF16, tag="xg", bufs=3)
                        nc.gpsimd.indirect_dma_start(
                            out=xg[:], out_offset=None, in_=xb[:, :],
                            in_offset=bass.IndirectOffsetOnAxis(
                                ap=tok_i[:, ct:ct + 1], axis=0))
                        if ct < 2:
                            for k in range(NK):
                                tp = pf.tile([128, 128], BF16, tag="ptr", bufs=1)
                                nc.tensor.transpose(
                                    out=tp[:], in_=xg[:, k * 128:(k + 1) * 128],
                                    identity=idb_sb[:])
                                nc.vector.tensor_copy(
                                    xte[:, k, ct * 128:(ct + 1) * 128], tp[:])
                        else:
                            nc.scalar.dma_start_transpose(
                                xte[:, :, ct * 128:(ct + 1) * 128], xg[:])

                    # mm1 + GELU: hT[f] = gelu(W1[:,f].T @ X.T + b1[f])
                    ht = [fp.tile([128, C], BF16, tag=f"ht{f}", name=f"ht{f}")
                          for f in range(NF)]
                    for f in range(NF):
                        w1c = fp.tile([128, NK * 128], BF16, tag="w1c", bufs=4)
                        nc.sync.dma_start(w1c[:], w1h[e, f])
                        psA = pf.tile([128, 320], F32, tag="m1", bufs=2)
                        psB = pf.tile([128, 320], F32, tag="m1", bufs=2)
                        for k in range(NK):
                            lw = w1c[:, k * 128:(k + 1) * 128]
                            nc.tensor.matmul(psA[:], lhsT=lw, rhs=xte[:, k, 0:320],
                                             start=(k == 0), stop=(k == NK - 1))
                            nc.tensor.matmul(psB[:], lhsT=lw, rhs=xte[:, k, 320:640],
                                             start=(k == 0), stop=(k == NK - 1))
                        nc.scalar.activation(ht[f][:, 0:320], psA[:], ACTF.Gelu,
                                             bias=b1_sb[e][:, f:f + 1])
                        nc.scalar.activation(ht[f][:, 320:640], psB[:], ACTF.Gelu,
                                             bias=b1_sb[e][:, f:f + 1])

                    # mm2; raw Ye rows straight out (bias+gating on host)
                    for dd in range(ND):
                        psY = [pf.tile([128, 512], F32, tag=f"m2_{ct}",
                                       name=f"m2_{ct}", bufs=1) for ct in range(NCT)]
                        for f in range(NF):
                            w2c = fp.tile([128, 512], BF16, tag="w2c", bufs=6)
                            nc.sync.dma_start(w2c[:], w2h[e, dd, f])
                            for ct in range(NCT):
                                nc.tensor.matmul(psY[ct][:],
                                                 lhsT=ht[f][:, ct * 128:(ct + 1) * 128],
                                                 rhs=w2c[:],
                                                 start=(f == 0), stop=(f == NF - 1))
                        for ct in range(NCT):
                            yo = fp.tile([128, 512], F32, tag="yo", bufs=3)
                            nc.vector.tensor_copy(yo[:], psY[ct][:])
                            eng = nc.scalar if ct % 2 == 0 else nc.sync
                            eng.dma_start(
                                yeh[e, ct, :, dd * 512:(dd + 1) * 512], yo[:])

    nc.finalize()
    return nc


def _prep_inputs(x, Wg, W1, b1, W2, b2):
    x = np.asarray(x, np.float32).reshape(T, D)
    x8 = x.astype(ml_dtypes.bfloat16)
    xr = (x - x8.astype(np.float32)).astype(ml_dtypes.bfloat16)
    x8t = np.ascontiguousarray(
        x8.reshape(NCH, 512, NK, 128).transpose(0, 3, 2, 1))
    xrt = np.ascontiguousarray(
        xr.reshape(NCH, 512, NK, 128).transpose(0, 3, 2, 1))
    xb = np.vstack([x8, np.zeros((1, D), ml_dtypes.bfloat16)])

    Wg = np.asarray(Wg, np.float32)
    wg8 = Wg.astype(ml_dtypes.bfloat16)
    wgr = (Wg - wg8.astype(np.float32)).astype(ml_dtypes.bfloat16)
    wg8h = np.ascontiguousarray(
        wg8.reshape(NK, 128, E).transpose(1, 0, 2)).reshape(128, NK * E)
    wgrh = np.ascontiguousarray(
        wgr.reshape(NK, 128, E).transpose(1, 0, 2)).reshape(128, NK * E)

    W1 = np.asarray(W1, np.float32)
    W2 = np.asarray(W2, np.float32)
    b1 = np.asarray(b1, np.float32)

    tokids = (np.arange(NB, dtype=np.float32)[None, :] * 128
              + np.arange(128, dtype=np.float32)[:, None])
    iota16 = np.broadcast_to(np.arange(E, dtype=np.float32), (128, E)).copy()
    iotac = (np.arange(NCT, dtype=np.float32)[None, :] * 128
             + np.arange(128, dtype=np.float32)[:, None])
    tri128 = np.triu(np.ones((128, 128), np.float32)).astype(ml_dtypes.bfloat16)
    ident16 = np.eye(16, dtype=np.float32)
    onescol = np.ones((128, 1), ml_dtypes.bfloat16)
    onesrow = np.ones((1, 128), np.float32)
    identb = np.eye(128, dtype=np.float32).astype(ml_dtypes.bfloat16)

    in_maps = []
    for c in range(8):
        el = slice(2 * c, 2 * c + 2)
        w1h = np.ascontiguousarray(
            W1[el].reshape(EL, NK, 128, NF, 128).transpose(0, 3, 2, 1, 4)
        ).astype(ml_dtypes.bfloat16).reshape(EL, NF, 128, NK * 128)
        w2h = np.ascontiguousarray(
            W2[el].reshape(EL, NF, 128, ND, 512).transpose(0, 3, 1, 2, 4)
        ).astype(ml_dtypes.bfloat16)
        b1h = np.ascontiguousarray(b1[el].reshape(EL, NF, 128).transpose(0, 2, 1))
        basev = np.full((128, 1), float(EL * C) * c, np.float32)
        esel2 = np.zeros((128, EL * E), np.float32)
        esel2[:, 2 * c] = 1.0
        esel2[:, E + 2 * c + 1] = 1.0
        in_maps.append(dict(x8t=x8t, xrt=xrt, xb=xb, wg8h=wg8h, wgrh=wgrh,
                            w1h=w1h, w2h=w2h, b1h=b1h, tokids=tokids,
                            iota16=iota16, iotac=iotac, tri128=tri128,
                            ident16=ident16, onescol=onescol, onesrow=onesrow,
                            identb=identb, basev=basev, esel2=esel2))
    return in_maps


def _run(inputs, trace=False, trace_cores=None):
    if "nc" not in _CACHE:
        _CACHE["nc"] = _build_nc()
    nc = _CACHE["nc"]
    in_maps = _prep_inputs(inputs["x"], inputs["Wg"], inputs["W1"],
                           inputs["b1"], inputs["W2"], inputs["b2"])
    res = run_bass_kernel_spmd(nc, in_maps, list(range(8)), trace=trace,
                               trace_cores=trace_cores)
    b2 = np.asarray(inputs["b2"], np.float32)
    y = np.zeros((T + 1, D), np.float32)
    for cid, r in enumerate(res.results):
        ye = r["yeh"].reshape(EL, NCT * 128, ND * 512)       # [EL, C, D]
        sl = r["gsl"]                                        # [EL, 128, NCT, 2]
        for e in range(EL):
            eg = 2 * cid + e
            tok = sl[e, :, :, 0].T.reshape(-1).astype(np.int64)   # slot c = ct*128+p
            gate = sl[e, :, :, 1].T.reshape(-1)
            valid = tok < T
            idx = tok[valid]
            y[idx] += gate[valid, None] * (ye[e][valid] + b2[eg][None, :])
    return y[:T].reshape(B, S, D), res


def kernel(x, Wg, W1, b1, W2, b2):
    y, _ = _run(dict(x=x, Wg=Wg, W1=W1, b1=b1, W2=W2, b2=b2))
    return y

